# revision 1
# baseline (speedup 1.0000x reference)
"""GNN message-passing kernel for 8 Trainium2 NeuronCores — v2.

Math: 3 layers of
    m   = relu(relu(x[src] @ Wm1 + bm1) @ Wm2 + bm2)      # message MLP
    aggr= segment_sum(m, dst)                              # scatter-add
    x   = relu(relu([aggr, x] @ Wu1 + bu1) @ Wu2 + bu2)    # update MLP

Structure (dst-sharded edges, per-node message table):
  - nodes sharded 8 ways (12500/core); message MLP computed per-node.
  - the per-node message table is built in 4 QUARTERS; each quarter is
    AllGathered separately (separate DRAM tensors) so collectives overlap
    with the gather/segment-sum of earlier quarters.
  - per-edge work = dma_gather of 256B message rows (int16 chunk-relative
    indices) + segment-sum via PE matmul against a one-hot slot matrix.
  - a 32KB/partition SWDGE descriptor ring lets Q7 descriptor generation
    of gather k+1 overlap the DMA transfer of gather k (the Q7 descgen at
    ~8ns/index is the pacing resource).
  - the update MLP is interleaved into the last chunk's evacuations and the
    next layer's message MLP + AllGathers are emitted right after, so layer
    boundaries do not stall the gather pipeline.
"""

import sys

sys.path.insert(0, "/opt/trn_rl_repo")

import os
import numpy as np
import ml_dtypes

KLAYERS = 3
NQ = 1             # SWDGE queues
SUB_TILES = 64     # gather sub-call tiles
KSP = False        # single_packet (broken on this runtime)
KPREP = False      # prepare_only+trigger (no win measured)
KSCRATCH = 32768   # SWDGE ring bytes/partition (2 gathers in flight)
KIDXRES = False    # per-chunk idx loads (SBUF headroom)

N_NODES = 100000
N_EDGES = 1600000
D = 64
H = 16
N_LAYERS = 3
NCORES = 8

B = 12500          # real nodes per core
QR = 3125          # real nodes per quarter (original-id quarters)
QP = 3200          # packed quarter rows (25 x 128)
NQTR = 4           # quarters
CH = NCORES * QP   # 25600 table rows per chunk (= quarter)
PADIDX = 0         # pad gathers use sentinel slot 255, content irrelevant
BPD = NQTR * QP    # 12800 packed positions per core (100 x 128)
NSW = 100          # dst subwindows of 128
NWIN = 25          # dst windows of 512 (exact)

_BF16 = ml_dtypes.bfloat16


def _pack_nodes(edge_index):
    """Greedy vector bin-packing: per (core, quarter), assign its 3125 nodes
    to 25 subwindow bins of 128 slots, balancing per-src-quarter in-degrees.
    Returns pos[n] = packed position within the core (0..12799)."""
    src = np.asarray(edge_index[0], dtype=np.int64)
    dst = np.asarray(edge_index[1], dtype=np.int64)
    srcq = (src % B) // QR  # src quarter (invariant under packing)
    # per-node in-degree by src quarter: d[n, k]
    d = np.zeros((N_NODES, NQTR), dtype=np.int32)
    np.add.at(d, (dst, srcq), 1)

    pos = np.zeros(N_NODES, dtype=np.int64)
    for c in range(NCORES):
        for q in range(NQTR):
            nodes = np.arange(c * B + q * QR, c * B + (q + 1) * QR)
            dv = d[nodes].astype(np.int64)
            order = np.argsort(-dv.sum(axis=1), kind="stable")
            loads = np.zeros((25, NQTR), dtype=np.int64)
            fill = np.zeros(25, dtype=np.int64)
            binof = np.empty(QR, dtype=np.int64)
            for i in order:
                cand = fill < 128
                cost = (loads + dv[i]).max(axis=1)
                cost[~cand] = 1 << 60
                b = int(np.argmin(cost))
                binof[i] = b
                loads[b] += dv[i]
                fill[b] += 1
            # assign slots within bins in node order
            slot_ctr = np.zeros(25, dtype=np.int64)
            for i in range(QR):
                b = binof[i]
                pos[nodes[i]] = q * QP + b * 128 + slot_ctr[b]
                slot_ctr[b] += 1
    return pos


def _preprocess(edge_index):
    """Per-core padded gather-index and slot arrays (layer-invariant)."""
    src = np.asarray(edge_index[0], dtype=np.int64)
    dst = np.asarray(edge_index[1], dtype=np.int64)
    pos = _pack_nodes(edge_index)

    core = dst // B
    dp = pos[dst]          # packed position of dst within its core
    sw = dp >> 7
    slot = (dp & 127).astype(np.uint8)

    sc = src // B          # src core
    sp_ = pos[src]         # packed position of src
    q = sp_ // QP          # chunk (= src packed quarter)
    rel = (sc * QP + (sp_ - q * QP)).astype(np.int16)  # chunk-relative row

    key = ((core * NQTR + q) * NSW + sw).astype(np.int64)
    order = np.argsort(key, kind="stable")
    key_s = key[order]
    rel_s = rel[order]
    slot_s = slot[order]
    core_s = core[order]
    chunk_s = q[order]

    counts = np.bincount(key, minlength=NCORES * NQTR * NSW).reshape(
        NCORES, NQTR, NSW
    )
    T = np.maximum(1, -(-counts // 128)).max(axis=0)  # [NQTR, NSW]
    cap = T * 128

    grp_start = np.zeros(NCORES * NQTR * NSW, dtype=np.int64)
    grp_start[1:] = np.cumsum(counts.ravel())[:-1]
    within = np.arange(len(key_s), dtype=np.int64) - grp_start[key_s]

    base = np.zeros((NQTR, NSW), dtype=np.int64)
    base[:, 1:] = np.cumsum(cap, axis=1)[:, :-1]
    dest = base[chunk_s, key_s % NSW] + within

    Lk = cap.sum(axis=1).astype(np.int64)  # padded edge-slots per chunk
    idx_pads = []
    slot_pads = []
    for k in range(NQTR):
        ip = np.full((NCORES, Lk[k]), PADIDX, dtype=np.int16)
        # sentinel slot 255: pad one-hot columns compare false against
        # iota 0..127, so pad gathers contribute nothing (gathered garbage ok)
        sp = np.full((NCORES, Lk[k]), 255, dtype=np.uint8)
        m = chunk_s == k
        flat = core_s[m] * Lk[k] + dest[m]
        ip.reshape(-1)[flat] = rel_s[m]
        sp.reshape(-1)[flat] = slot_s[m]
        idx_pads.append(ip)
        slot_pads.append(sp)

    # wrap: idx position i -> [i%16, i//16]; replicate rows 0-15 into 16-31
    idx_wrapped = []
    slot_wrapped = []
    for k in range(NQTR):
        iw = np.zeros((NCORES, 128, Lk[k] // 16), dtype=np.int16)
        w = idx_pads[k].reshape(NCORES, -1, 16).transpose(0, 2, 1)
        iw[:, 0:16] = w
        iw[:, 16:32] = w
        idx_wrapped.append(iw)
        slot_wrapped.append(
            slot_pads[k]
            .reshape(NCORES, -1, 128)
            .transpose(0, 2, 1)
            .astype(_BF16)
        )
    idx_all = np.concatenate(idx_wrapped, axis=2)    # [8, 128, sum(Lk)/16]
    slot_all = np.concatenate(slot_wrapped, axis=2)  # [8, 128, sum(Lk)/128]
    return T, Lk, idx_all, slot_all, pos


def _build_program(T, Lk):
    import concourse.bass as bass
    import concourse.bacc as bacc
    import concourse.tile as tile
    from concourse import mybir

    dt = mybir.dt
    AF = mybir.ActivationFunctionType
    ALU = mybir.AluOpType

    nc = bacc.Bacc(
        "TRN2", target_bir_lowering=False, debug=False, num_devices=NCORES,
        num_swdge_queues=NQ, dynamic_dma_scratch_size=KSCRATCH,
    )

    # ---- I/O ----
    x0_d = nc.dram_tensor("x0", [BPD, D], dt.float32, kind="ExternalInput")
    idx_cols = int(Lk.sum()) // 16
    slot_cols = int(Lk.sum()) // 128
    idx_d = nc.dram_tensor("idx", [128, idx_cols], dt.int16, kind="ExternalInput")
    slot_d = nc.dram_tensor(
        "slot", [128, slot_cols], dt.bfloat16, kind="ExternalInput"
    )
    wm1_d = nc.dram_tensor("wm1", [N_LAYERS, D, H], dt.bfloat16, kind="ExternalInput")
    bm1_d = nc.dram_tensor("bm1", [N_LAYERS, H, 1], dt.float32, kind="ExternalInput")
    wm2_d = nc.dram_tensor("wm2", [N_LAYERS, H, D], dt.bfloat16, kind="ExternalInput")
    bm2_d = nc.dram_tensor("bm2", [N_LAYERS, D, 1], dt.float32, kind="ExternalInput")
    wu1a_d = nc.dram_tensor("wu1a", [N_LAYERS, D, H], dt.float32, kind="ExternalInput")
    wu1b_d = nc.dram_tensor(
        "wu1b", [N_LAYERS, D, H], dt.bfloat16, kind="ExternalInput"
    )
    bu1_d = nc.dram_tensor("bu1", [N_LAYERS, H, 1], dt.float32, kind="ExternalInput")
    wu2_d = nc.dram_tensor("wu2", [N_LAYERS, H, D], dt.bfloat16, kind="ExternalInput")
    bu2_d = nc.dram_tensor("bu2", [N_LAYERS, D, 1], dt.float32, kind="ExternalInput")
    iota_d = nc.dram_tensor("iota", [128, 128], dt.bfloat16, kind="ExternalInput")
    idenf_d = nc.dram_tensor("idenf", [128, 128], dt.float32, kind="ExternalInput")
    idenb_d = nc.dram_tensor("idenb", [64, 64], dt.bfloat16, kind="ExternalInput")
    out_d = nc.dram_tensor("out", [BPD, D], dt.float32, kind="ExternalOutput")

    # per-quarter message blocks and gathered tables
    block_q = [
        nc.dram_tensor(f"mblock{q}", [QP, 128], dt.bfloat16) for q in range(NQTR)
    ]
    table_q = [
        nc.dram_tensor(f"mtable{q}", [CH, 128], dt.bfloat16, addr_space="Shared")
        for q in range(NQTR)
    ]

    # per-chunk tile schedule
    sw_of_tile = []
    tflag = []
    for k in range(NQTR):
        sws = []
        fl = []
        for sw in range(NSW):
            t = int(T[k][sw])
            for j in range(t):
                sws.append(sw)
                fl.append((j == 0, j == t - 1))
        sw_of_tile.append(sws)
        tflag.append(fl)
    tiles_k = [len(s) for s in sw_of_tile]
    idx_off16 = np.concatenate([[0], np.cumsum(Lk // 16)]).astype(int)
    slot_offT = np.concatenate([[0], np.cumsum(Lk // 128)]).astype(int)

    from contextlib import ExitStack

    with tile.TileContext(nc) as tc, ExitStack() as ctx:
        P = lambda **kw: ctx.enter_context(tc.tile_pool(**kw))
        cpool = P(name="consts", bufs=1)
        xpool = P(name="xT", bufs=1)
        apool = P(name="aggr", bufs=1)
        slpool = P(name="slot", bufs=1)
        idxpool = P(name="idx", bufs=1 if KIDXRES else 2)
        gpool = P(name="G", bufs=2)
        spool = P(name="S", bufs=2)
        mpool = P(name="mT", bufs=2)
        hpool = P(name="hid", bufs=2)
        stgpool = P(name="stg", bufs=3)
        xldpool = P(name="xld", bufs=2)
        xopool = P(name="xout", bufs=2)
        ps_seg = P(name="ps_seg", bufs=2, space="PSUM")
        ps_m16 = P(name="ps_m16", bufs=2, space="PSUM")
        ps_m64 = P(name="ps_m64", bufs=2, space="PSUM")
        ps_tr = P(name="ps_tr", bufs=2, space="PSUM")

        # ---- constants ----
        iota_t = cpool.tile([128, 128], dt.bfloat16, tag="iota")
        nc.sync.dma_start(iota_t[:], iota_d[:])
        idenf_t = cpool.tile([128, 128], dt.float32, tag="idenf")
        nc.sync.dma_start(idenf_t[:], idenf_d[:])
        idenb_t = cpool.tile([64, 64], dt.bfloat16, tag="idenb")
        nc.sync.dma_start(idenb_t[:], idenb_d[:])

        def _w(name, dram, shape, dtype):
            ts = []
            for l in range(N_LAYERS):
                t = cpool.tile(shape, dtype, tag=f"{name}{l}")
                nc.sync.dma_start(t[:], dram[l])
                ts.append(t)
            return ts

        wm1_t = _w("wm1", wm1_d, [D, H], dt.bfloat16)
        bm1_t = _w("bm1", bm1_d, [H, 1], dt.float32)
        wm2_t = _w("wm2", wm2_d, [H, D], dt.bfloat16)
        bm2_t = _w("bm2", bm2_d, [D, 1], dt.float32)
        wu1a_t = _w("wu1a", wu1a_d, [D, H], dt.float32)
        wu1b_t = _w("wu1b", wu1b_d, [D, H], dt.bfloat16)
        bu1_t = _w("bu1", bu1_d, [H, 1], dt.float32)
        wu2_t = _w("wu2", wu2_d, [H, D], dt.bfloat16)
        bu2_t = _w("bu2", bu2_d, [D, 1], dt.float32)

        slot_t = slpool.tile([128, slot_cols], dt.bfloat16)
        nc.sync.dma_start(slot_t[:], slot_d[:])
        if KIDXRES:
            idx_res = idxpool.tile([128, idx_cols], dt.int16)
            nc.sync.dma_start(idx_res[:], idx_d[:])

        # ---- x0 -> x^T (bf16) ----  (dst layout: BPD cols)
        xA = xpool.tile([D, BPD], dt.bfloat16, tag="xA")
        for t in range(NSW):
            xld = xldpool.tile([128, D], dt.float32)
            nc.sync.dma_start(xld[:], x0_d[128 * t: 128 * (t + 1), :])
            pt = ps_tr.tile([D, 128], dt.float32)
            nc.tensor.transpose(pt[:], xld[:], idenf_t[:])
            nc.scalar.activation(xA[:, 128 * t: 128 * (t + 1)], pt[:], AF.Copy)

        aggr = apool.tile([D, BPD], dt.float32)

        dma_sem = nc.alloc_semaphore("gather_dma") if KPREP else None
        gcount = [0]

        def msg_mlp_and_ag(layer):
            """Message MLP over all packed positions + per-quarter AllGather."""
            xT = xA
            for q in range(NQTR):
                xoff = q * QP
                wins = [(o, min(512, QP - o)) for o in range(0, QP, 512)]
                for (o, wd) in wins:
                    p1 = ps_m16.tile([H, 512], dt.float32)
                    nc.tensor.matmul(
                        p1[:, :wd], lhsT=wm1_t[layer][:],
                        rhs=xT[:, xoff + o: xoff + o + wd],
                        start=True, stop=True,
                    )
                    hid = hpool.tile([H, 512], dt.bfloat16, tag="hid")
                    nc.scalar.activation(
                        p1_out(hid, wd), p1[:, :wd], AF.Relu, bias=bm1_t[layer][:]
                    )
                    p2 = ps_m64.tile([D, 512], dt.float32)
                    nc.tensor.matmul(
                        p2[:, :wd], lhsT=wm2_t[layer][:], rhs=hid[:, :wd],
                        start=True, stop=True,
                    )
                    mt = mpool.tile([D, 512], dt.bfloat16)
                    nc.scalar.activation(
                        mt[:, :wd], p2[:, :wd], AF.Relu, bias=bm2_t[layer][:]
                    )
                    for st in range(-(-wd // 128)):
                        so = 128 * st
                        pt = ps_tr.tile([128, D], dt.bfloat16)
                        nc.tensor.transpose(
                            pt[:], mt[:, so: so + 128], idenb_t[:]
                        )
                        stg = stgpool.tile([128, D], dt.bfloat16, tag="stgb")
                        nc.scalar.activation(stg[:], pt[:], AF.Copy)
                        nc.sync.dma_start(
                            block_q[q][o + so: o + so + 128, 0:D], stg[:]
                        )
                nc.gpsimd.collective_compute(
                    "AllGather",
                    mybir.AluOpType.bypass,
                    replica_groups=[list(range(NCORES))],
                    ins=[block_q[q][:].opt()],
                    outs=[table_q[q][:].opt()],
                )

        def p1_out(hid, wd):
            return hid[:, :wd]

        def emit_update(layer, ch):
            """Update MLP for one 512-col window (aggr complete for it)."""
            xT = xA
            xN = xA
            o = 512 * ch
            wd = 512
            p1 = ps_m16.tile([H, 512], dt.float32)
            nc.tensor.matmul(
                p1[:, :wd], lhsT=wu1a_t[layer][:], rhs=aggr[:, o: o + wd],
                start=True, stop=False, skip_group_check=True,
            )
            nc.tensor.matmul(
                p1[:, :wd], lhsT=wu1b_t[layer][:], rhs=xT[:, o: o + wd],
                start=False, stop=True, skip_group_check=True,
            )
            hid = hpool.tile([H, 512], dt.bfloat16, tag="hid")
            nc.scalar.activation(
                hid[:, :wd], p1[:, :wd], AF.Relu, bias=bu1_t[layer][:]
            )
            p2 = ps_m64.tile([D, 512], dt.float32)
            nc.tensor.matmul(
                p2[:, :wd], lhsT=wu2_t[layer][:], rhs=hid[:, :wd],
                start=True, stop=True,
            )
            if layer < N_LAYERS - 1:
                nc.scalar.activation(
                    xN[:, o: o + wd], p2[:, :wd], AF.Relu,
                    bias=bu2_t[layer][:],
                )
            else:
                xo = xopool.tile([D, 512], dt.float32)
                nc.scalar.activation(
                    xo[:, :wd], p2[:, :wd], AF.Relu, bias=bu2_t[layer][:]
                )
                for st in range(-(-wd // 128)):
                    so = 128 * st
                    tw = min(128, wd - so)
                    pt = ps_tr.tile([128, D], dt.float32)
                    nc.tensor.transpose(
                        pt[:tw, :], xo[:, so: so + tw], idenf_t[0:D, 0:D]
                    )
                    stg = stgpool.tile([128, D], dt.float32, tag="stgf")
                    nc.scalar.activation(stg[:tw, :], pt[:tw, :], AF.Copy)
                    nc.sync.dma_start(
                        out_d[o + so: o + so + tw, :], stg[:tw, :]
                    )

        msg_mlp_and_ag(0)
        for layer in range(KLAYERS):

            # ---- gather + segment matmul passes (chunk = quarter) ----
            for k in range(NQTR):
                if KIDXRES:
                    idx_t = idx_res[:, idx_off16[k]: idx_off16[k + 1]]
                else:
                    nck = int(Lk[k]) // 16
                    idx_ld = idxpool.tile([128, nck], dt.int16, tag="idxld")
                    nc.sync.dma_start(
                        idx_ld[:], idx_d[:, idx_off16[k]: idx_off16[k] + nck]
                    )
                    idx_t = idx_ld[:]
                ntiles = tiles_k[k]
                nsub = -(-ntiles // SUB_TILES)
                cur_win = -1
                pw = None
                tb = 0
                for si in range(nsub):
                    nt = min(SUB_TILES, ntiles - tb)
                    g = gpool.tile([128, SUB_TILES, 128], dt.bfloat16)
                    nidx = nt * 128
                    idx_sl = idx_t[:, 8 * tb: 8 * (tb + nt)]
                    if KPREP:
                        nc.gpsimd.dma_gather(
                            g[:, :nt, :], table_q[k][:], idx_sl, nidx, nidx, 128,
                            single_packet=KSP, prepare_only=True, sem=dma_sem,
                        )
                        nc.gpsimd.trigger_dma(count=None)
                        gcount[0] += 1
                        nc.tensor.wait_ge(dma_sem, 16 * gcount[0])
                    else:
                        nc.gpsimd.dma_gather(
                            g[:, :nt, :], table_q[k][:], idx_sl, nidx, nidx, 128,
                            single_packet=KSP,
                            queue_num=(k * 1000 + si) % NQ if NQ > 1 else 0,
                        )
                    sb = spool.tile([128, SUB_TILES, 128], dt.bfloat16)
                    io_b = iota_t[:].unsqueeze(1).broadcast_to([128, nt, 128])
                    sl_b = (
                        slot_t[:, slot_offT[k] + tb: slot_offT[k] + tb + nt]
                        .unsqueeze(2)
                        .broadcast_to([128, nt, 128])
                    )
                    nc.vector.tensor_tensor(sb[:, :nt, :], io_b, sl_b, ALU.is_equal)

                    for j in range(nt):
                        ti = tb + j
                        sw = sw_of_tile[k][ti]
                        first, last = tflag[k][ti]
                        win = sw // 4
                        if win != cur_win:
                            if pw is not None:
                                _evac(nc, ALU, aggr, pw, cur_win, k)
                                if k == NQTR - 1:
                                    emit_update(layer, cur_win)
                            pw = ps_seg.tile([D, 512], dt.float32)
                            cur_win = win
                        nc.tensor.matmul(
                            pw[:, 128 * (sw % 4): 128 * (sw % 4) + 128],
                            lhsT=g[:, j, 0:D],
                            rhs=sb[:, j, :],
                            start=first,
                            stop=last,
                        )
                    tb += nt
                if pw is not None:
                    _evac(nc, ALU, aggr, pw, cur_win, k)
                    if k == NQTR - 1:
                        emit_update(layer, cur_win)

            if layer < KLAYERS - 1:
                msg_mlp_and_ag(layer + 1)

    nc.finalize()
    return nc


def _evac(nc, ALU, aggr, pw, win, k):
    o = 512 * win
    wd = min(512, BPD - o)
    if k == 0:
        nc.vector.tensor_copy(aggr[:, o: o + wd], pw[:, :wd])
    else:
        nc.vector.tensor_tensor(
            aggr[:, o: o + wd], pw[:, :wd], aggr[:, o: o + wd], ALU.add
        )


_CACHE = {}


def _make_in_maps(inputs, idx_all, slot_all, pos):
    x = np.asarray(inputs["x"], dtype=np.float32)
    Wm1, bm1 = inputs["Wm1"], inputs["bm1"]
    Wm2, bm2 = inputs["Wm2"], inputs["bm2"]
    Wu1, bu1 = inputs["Wu1"], inputs["bu1"]
    Wu2, bu2 = inputs["Wu2"], inputs["bu2"]

    xpad = np.zeros((NCORES, BPD, D), dtype=np.float32)
    for c in range(NCORES):
        nodes = np.arange(c * B, (c + 1) * B)
        xpad[c, pos[nodes]] = x[nodes]

    iota = np.broadcast_to(np.arange(128, dtype=np.float32), (128, 128)).astype(_BF16)
    idenf = np.eye(128, dtype=np.float32)
    idenb = np.eye(64, dtype=np.float32).astype(_BF16)

    com = {
        "wm1": np.ascontiguousarray(np.asarray(Wm1, np.float32)).astype(_BF16),
        "bm1": np.asarray(bm1, np.float32).reshape(N_LAYERS, H, 1),
        "wm2": np.ascontiguousarray(np.asarray(Wm2, np.float32)).astype(_BF16),
        "bm2": np.asarray(bm2, np.float32).reshape(N_LAYERS, D, 1),
        "wu1a": np.ascontiguousarray(np.asarray(Wu1, np.float32)[:, :D, :]),
        "wu1b": np.ascontiguousarray(np.asarray(Wu1, np.float32)[:, D:, :]).astype(
            _BF16
        ),
        "bu1": np.asarray(bu1, np.float32).reshape(N_LAYERS, H, 1),
        "wu2": np.ascontiguousarray(np.asarray(Wu2, np.float32)).astype(_BF16),
        "bu2": np.asarray(bu2, np.float32).reshape(N_LAYERS, D, 1),
        "iota": iota,
        "idenf": idenf,
        "idenb": idenb,
    }
    in_maps = []
    for c in range(NCORES):
        m = dict(com)
        m["x0"] = xpad[c]
        m["idx"] = np.ascontiguousarray(idx_all[c])
        m["slot"] = np.ascontiguousarray(slot_all[c])
        in_maps.append(m)
    return in_maps


def kernel(x, edge_index, Wm1, bm1, Wm2, bm2, Wu1, bu1, Wu2, bu2):
    from concourse.bass_utils import run_bass_kernel_spmd

    ei_key = hash(np.asarray(edge_index)[:, ::97].tobytes())
    if ei_key not in _CACHE:
        T, Lk, idx_all, slot_all, pos = _preprocess(edge_index)
        nc = _build_program(T, Lk)
        _CACHE[ei_key] = (nc, idx_all, slot_all, pos)
    nc, idx_all, slot_all, pos = _CACHE[ei_key]

    in_maps = _make_in_maps(
        dict(
            x=x, edge_index=edge_index, Wm1=Wm1, bm1=bm1, Wm2=Wm2, bm2=bm2,
            Wu1=Wu1, bu1=bu1, Wu2=Wu2, bu2=bu2,
        ),
        idx_all,
        slot_all,
        pos,
    )
    res = run_bass_kernel_spmd(nc, in_maps, core_ids=list(range(NCORES)))
    out = np.empty((N_NODES, D), dtype=np.float32)
    for c in range(NCORES):
        nodes = np.arange(c * B, (c + 1) * B)
        out[nodes] = res.results[c]["out"][pos[nodes]]
    return out


if __name__ == "__main__":
    import reference

    inputs = reference.setup_inputs()
    inputs = {k: np.asarray(v) for k, v in inputs.items()}
    got = kernel(**inputs)
    exp = np.asarray(reference.reference(**{k: v for k, v in inputs.items()}))
    err = np.abs(got - exp)
    rel = np.linalg.norm(got - exp) / np.linalg.norm(exp)
    print("max abs err:", err.max(), "rel:", rel)



# revision 3
# speedup vs baseline: 1.0652x; 1.0652x over previous
"""GNN message-passing kernel for 8 Trainium2 NeuronCores — v3.

Math: 3 layers of
    m   = relu(relu(x[src] @ Wm1 + bm1) @ Wm2 + bm2)      # message MLP
    aggr= segment_sum(m, dst)                              # scatter-add
    x   = relu(relu([aggr, x] @ Wu1 + bu1) @ Wu2 + bu2)    # update MLP

v6 over v5: deferred-chunk-3 schedule — the layer walks (window-pair,
chunk) blocks with chunk 3 lagging DELTA pairs, so aggr windows complete
(and updates + next-layer MLP/AllGathers issue) spread across the whole
layer instead of clustering at its end; AllGather tails no longer stall
the next layer's chunk starts.

v5 over v4: x0 supplied host-transposed ([D, BPD]) so the prologue needs
no PE transposes; the output is returned transposed ([D, BPD]) and
re-transposed on host, removing the tail transpose chain; gather pool
deepened to 7.

v4 over v3: resident idx (no per-chunk loads), deeper gather buffers.

v3 over v2:
  - 4 SWDGE queues: dma_gather descgen runs on all 8 Pool Q7 CPUs (one
    TX/RX pair per queue, instructions pipeline across queues) -> measured
    2.5ns/idx vs 9.3ns/idx single-queue.  Queue q's Q7 pair reads its idx
    slice from partitions 32q..32q+31, so the wrapped idx array is
    replicated into all eight 16-partition groups.
  - smaller gather sub-calls (32 tiles) with 4 outstanding buffers so the
    four queues stay fed.
  - the next layer's message MLP + per-quarter AllGather are emitted as
    soon as the update windows covering that quarter complete (during the
    last gather chunk), hiding MLP+collective latency under the gathers.
  - x0 prologue is processed per quarter so AllGather(q0) issues early.
"""

import sys

sys.path.insert(0, "/opt/trn_rl_repo")

import numpy as np
import ml_dtypes

KLAYERS = 3
NQ = 4             # SWDGE queues
SUB_TILES = 32     # gather sub-call tiles
KSCRATCH = 32768   # SWDGE ring bytes/partition
DELTA = 4          # chunk-3 deferral in window-pairs

N_NODES = 100000
N_EDGES = 1600000
D = 64
H = 16
N_LAYERS = 3
NCORES = 8

B = 12500          # real nodes per core
QR = 3125          # real nodes per quarter (original-id quarters)
QP = 3200          # packed quarter rows (25 x 128)
NQTR = 4           # quarters
CH = NCORES * QP   # 25600 table rows per chunk (= quarter)
PADIDX = 0         # pad gathers use sentinel slot 255, content irrelevant
BPD = NQTR * QP    # 12800 packed positions per core (100 x 128)
NSW = 100          # dst subwindows of 128
NWIN = 25          # dst windows of 512 (exact)

_BF16 = ml_dtypes.bfloat16


def _pack_nodes(edge_index):
    """Greedy vector bin-packing: per (core, quarter), assign its 3125 nodes
    to 25 subwindow bins of 128 slots, balancing per-src-quarter in-degrees.
    Returns pos[n] = packed position within the core (0..12799)."""
    src = np.asarray(edge_index[0], dtype=np.int64)
    dst = np.asarray(edge_index[1], dtype=np.int64)
    srcq = (src % B) // QR  # src quarter (invariant under packing)
    d = np.zeros((N_NODES, NQTR), dtype=np.int32)
    np.add.at(d, (dst, srcq), 1)

    pos = np.zeros(N_NODES, dtype=np.int64)
    for c in range(NCORES):
        for q in range(NQTR):
            nodes = np.arange(c * B + q * QR, c * B + (q + 1) * QR)
            dv = d[nodes].astype(np.int64)
            order = np.argsort(-dv.sum(axis=1), kind="stable")
            loads = np.zeros((25, NQTR), dtype=np.int64)
            fill = np.zeros(25, dtype=np.int64)
            binof = np.empty(QR, dtype=np.int64)
            for i in order:
                cand = fill < 128
                cost = (loads + dv[i]).max(axis=1)
                cost[~cand] = 1 << 60
                b = int(np.argmin(cost))
                binof[i] = b
                loads[b] += dv[i]
                fill[b] += 1
            slot_ctr = np.zeros(25, dtype=np.int64)
            for i in range(QR):
                b = binof[i]
                pos[nodes[i]] = q * QP + b * 128 + slot_ctr[b]
                slot_ctr[b] += 1
    return pos


def _preprocess(edge_index):
    """Per-core padded gather-index and slot arrays (layer-invariant)."""
    src = np.asarray(edge_index[0], dtype=np.int64)
    dst = np.asarray(edge_index[1], dtype=np.int64)
    pos = _pack_nodes(edge_index)

    core = dst // B
    dp = pos[dst]          # packed position of dst within its core
    sw = dp >> 7
    slot = (dp & 127).astype(np.uint8)

    sc = src // B          # src core
    sp_ = pos[src]         # packed position of src
    q = sp_ // QP          # chunk (= src packed quarter)
    rel = (sc * QP + (sp_ - q * QP)).astype(np.int16)  # chunk-relative row

    key = ((core * NQTR + q) * NSW + sw).astype(np.int64)
    order = np.argsort(key, kind="stable")
    key_s = key[order]
    rel_s = rel[order]
    slot_s = slot[order]
    core_s = core[order]
    chunk_s = q[order]

    counts = np.bincount(key, minlength=NCORES * NQTR * NSW).reshape(
        NCORES, NQTR, NSW
    )
    T = np.maximum(1, -(-counts // 128)).max(axis=0)  # [NQTR, NSW]
    cap = T * 128

    grp_start = np.zeros(NCORES * NQTR * NSW, dtype=np.int64)
    grp_start[1:] = np.cumsum(counts.ravel())[:-1]
    within = np.arange(len(key_s), dtype=np.int64) - grp_start[key_s]

    base = np.zeros((NQTR, NSW), dtype=np.int64)
    base[:, 1:] = np.cumsum(cap, axis=1)[:, :-1]
    dest = base[chunk_s, key_s % NSW] + within

    Lk = cap.sum(axis=1).astype(np.int64)  # padded edge-slots per chunk
    idx_pads = []
    slot_pads = []
    for k in range(NQTR):
        ip = np.full((NCORES, Lk[k]), PADIDX, dtype=np.int16)
        # sentinel slot 255: pad one-hot columns compare false against
        # iota 0..127, so pad gathers contribute nothing
        sp = np.full((NCORES, Lk[k]), 255, dtype=np.uint8)
        m = chunk_s == k
        flat = core_s[m] * Lk[k] + dest[m]
        ip.reshape(-1)[flat] = rel_s[m]
        sp.reshape(-1)[flat] = slot_s[m]
        idx_pads.append(ip)
        slot_pads.append(sp)

    # deferred-chunk-3 schedule: blocks of (chunk, window-pair).  Steps
    # s=0..12 emit pair s for chunks 0,1,2; chunk 3 of pair s-DELTA lags,
    # followed by that pair's update windows (action=pair index).
    tile_base = np.zeros((NQTR, NSW), dtype=np.int64)
    tile_base[:, 1:] = np.cumsum(T, axis=1)[:, :-1]

    def tiles_of(k, sws):
        out = []
        for sw in sws:
            t = int(T[k][sw])
            for j in range(t):
                out.append((k, sw, j == 0, j == t - 1, tile_base[k][sw] + j))
        return out

    blocks = []  # (chunk, tiles, action_pair_or_None)
    for s_ in range(13 + DELTA):
        if s_ <= 12:
            for k in range(3):
                blocks.append((k, tiles_of(k, range(8 * s_, min(8 * s_ + 8, NSW))), None))
        if s_ >= DELTA:
            p = s_ - DELTA
            blocks.append((3, tiles_of(3, range(8 * p, min(8 * p + 8, NSW))), p))

    # global idx/slot streams in schedule order
    order = []
    for (k, tiles, _a) in blocks:
        for (kk, sw, fi, la, tix) in tiles:
            order.append((kk, tix))
    n_tiles = len(order)
    idx_g = np.empty((NCORES, n_tiles * 128), dtype=np.int16)
    slot_g = np.empty((NCORES, n_tiles * 128), dtype=np.uint8)
    for i, (kk, tix) in enumerate(order):
        idx_g[:, 128 * i: 128 * (i + 1)] = idx_pads[kk][:, 128 * tix: 128 * (tix + 1)]
        slot_g[:, 128 * i: 128 * (i + 1)] = slot_pads[kk][:, 128 * tix: 128 * (tix + 1)]

    # wrap: idx position i -> [i%16, i//16]; replicate into all eight
    # 16-partition groups (queue q's Q7 pair reads partitions 32q..32q+31)
    iw = np.zeros((NCORES, 128, idx_g.shape[1] // 16), dtype=np.int16)
    w = idx_g.reshape(NCORES, -1, 16).transpose(0, 2, 1)
    for g in range(8):
        iw[:, 16 * g: 16 * (g + 1)] = w
    idx_all = iw
    slot_all = slot_g.reshape(NCORES, -1, 128).transpose(0, 2, 1).astype(_BF16)
    return T, blocks, idx_all, slot_all, pos


def _build_program(T, blocks):
    import concourse.bass as bass
    import concourse.bacc as bacc
    import concourse.tile as tile
    from concourse import mybir

    dt = mybir.dt
    AF = mybir.ActivationFunctionType
    ALU = mybir.AluOpType

    nc = bacc.Bacc(
        "TRN2", target_bir_lowering=False, debug=False, num_devices=NCORES,
        num_swdge_queues=NQ, dynamic_dma_scratch_size=KSCRATCH,
    )

    # ---- I/O ----
    x0_d = nc.dram_tensor("x0", [D, BPD], dt.float32, kind="ExternalInput")
    n_tiles_tot = sum(len(t) for (_k, t, _a) in blocks)
    idx_cols = n_tiles_tot * 8
    slot_cols = n_tiles_tot
    idx_d = nc.dram_tensor("idx", [128, idx_cols], dt.int16, kind="ExternalInput")
    slot_d = nc.dram_tensor(
        "slot", [128, slot_cols], dt.bfloat16, kind="ExternalInput"
    )
    wm1_d = nc.dram_tensor("wm1", [N_LAYERS, D, H], dt.bfloat16, kind="ExternalInput")
    bm1_d = nc.dram_tensor("bm1", [N_LAYERS, H, 1], dt.float32, kind="ExternalInput")
    wm2_d = nc.dram_tensor("wm2", [N_LAYERS, H, D], dt.bfloat16, kind="ExternalInput")
    bm2_d = nc.dram_tensor("bm2", [N_LAYERS, D, 1], dt.float32, kind="ExternalInput")
    wu1a_d = nc.dram_tensor("wu1a", [N_LAYERS, D, H], dt.float32, kind="ExternalInput")
    wu1b_d = nc.dram_tensor(
        "wu1b", [N_LAYERS, D, H], dt.bfloat16, kind="ExternalInput"
    )
    bu1_d = nc.dram_tensor("bu1", [N_LAYERS, H, 1], dt.float32, kind="ExternalInput")
    wu2_d = nc.dram_tensor("wu2", [N_LAYERS, H, D], dt.bfloat16, kind="ExternalInput")
    bu2_d = nc.dram_tensor("bu2", [N_LAYERS, D, 1], dt.float32, kind="ExternalInput")
    iota_d = nc.dram_tensor("iota", [128, 128], dt.bfloat16, kind="ExternalInput")
    idenf_d = nc.dram_tensor("idenf", [128, 128], dt.float32, kind="ExternalInput")
    idenb_d = nc.dram_tensor("idenb", [64, 64], dt.bfloat16, kind="ExternalInput")
    out_d = nc.dram_tensor("out", [D, BPD], dt.float32, kind="ExternalOutput")

    # per-quarter message blocks and gathered tables
    block_q = [
        nc.dram_tensor(f"mblock{q}", [QP, 128], dt.bfloat16) for q in range(NQTR)
    ]
    table_q = [
        [
            nc.dram_tensor(f"mtable{q}_{par}", [CH, 128], dt.bfloat16,
                           addr_space="Shared")
            for q in range(NQTR)
        ]
        for par in range(2)
    ]

    from contextlib import ExitStack

    with tile.TileContext(nc) as tc, ExitStack() as ctx:
        P = lambda **kw: ctx.enter_context(tc.tile_pool(**kw))
        cpool = P(name="consts", bufs=1)
        xpool = P(name="xT", bufs=1)
        apool = P(name="aggr", bufs=1)
        slpool = P(name="slot", bufs=1)
        gpool = P(name="G", bufs=7)
        spool = P(name="S", bufs=2)
        mpool = P(name="mT", bufs=2)
        hpool = P(name="hid", bufs=2)
        stgpool = P(name="stg", bufs=3)
        xldpool = P(name="xld", bufs=2)
        xopool = P(name="xout", bufs=1)
        ps_seg = P(name="ps_seg", bufs=2, space="PSUM")
        ps_m16 = P(name="ps_m16", bufs=2, space="PSUM")
        ps_m64 = P(name="ps_m64", bufs=2, space="PSUM")
        ps_tr = P(name="ps_tr", bufs=2, space="PSUM")

        # ---- constants ----
        iota_t = cpool.tile([128, 128], dt.bfloat16, tag="iota")
        nc.sync.dma_start(iota_t[:], iota_d[:])
        idenf_t = cpool.tile([128, 128], dt.float32, tag="idenf")
        nc.sync.dma_start(idenf_t[:], idenf_d[:])
        idenb_t = cpool.tile([64, 64], dt.bfloat16, tag="idenb")
        nc.sync.dma_start(idenb_t[:], idenb_d[:])

        def _w(name, dram, shape, dtype):
            ts = []
            for l in range(N_LAYERS):
                t = cpool.tile(shape, dtype, tag=f"{name}{l}")
                nc.sync.dma_start(t[:], dram[l])
                ts.append(t)
            return ts

        wm1_t = _w("wm1", wm1_d, [D, H], dt.bfloat16)
        bm1_t = _w("bm1", bm1_d, [H, 1], dt.float32)
        wm2_t = _w("wm2", wm2_d, [H, D], dt.bfloat16)
        bm2_t = _w("bm2", bm2_d, [D, 1], dt.float32)
        wu1a_t = _w("wu1a", wu1a_d, [D, H], dt.float32)
        wu1b_t = _w("wu1b", wu1b_d, [D, H], dt.bfloat16)
        bu1_t = _w("bu1", bu1_d, [H, 1], dt.float32)
        wu2_t = _w("wu2", wu2_d, [H, D], dt.bfloat16)
        bu2_t = _w("bu2", bu2_d, [D, 1], dt.float32)

        slot_t = slpool.tile([128, slot_cols], dt.bfloat16)
        nc.sync.dma_start(slot_t[:], slot_d[:])
        idx_res = slpool.tile([128, idx_cols], dt.int16, tag="idxres")
        nc.sync.dma_start(idx_res[:], idx_d[:])

        # ---- x0 -> x^T (bf16), per quarter; MLP+AG issued per quarter ----
        xA = xpool.tile([D, BPD], dt.bfloat16, tag="xA")
        aggr = apool.tile([D, BPD], dt.float32)

        def msg_mlp_quarter(layer, q):
            """Message MLP for quarter q of the packed positions + AllGather."""
            par = layer % 2
            xT = xA
            xoff = q * QP
            wins = [(o, min(512, QP - o)) for o in range(0, QP, 512)]
            for (o, wd) in wins:
                p1 = ps_m16.tile([H, 512], dt.float32)
                nc.tensor.matmul(
                    p1[:, :wd], lhsT=wm1_t[layer][:],
                    rhs=xT[:, xoff + o: xoff + o + wd],
                    start=True, stop=True,
                )
                hid = hpool.tile([H, 512], dt.bfloat16, tag="hid")
                nc.scalar.activation(
                    hid[:, :wd], p1[:, :wd], AF.Relu, bias=bm1_t[layer][:]
                )
                p2 = ps_m64.tile([D, 512], dt.float32)
                nc.tensor.matmul(
                    p2[:, :wd], lhsT=wm2_t[layer][:], rhs=hid[:, :wd],
                    start=True, stop=True,
                )
                mt = mpool.tile([D, 512], dt.bfloat16)
                nc.scalar.activation(
                    mt[:, :wd], p2[:, :wd], AF.Relu, bias=bm2_t[layer][:]
                )
                for st in range(-(-wd // 128)):
                    so = 128 * st
                    pt = ps_tr.tile([128, D], dt.bfloat16)
                    nc.tensor.transpose(
                        pt[:], mt[:, so: so + 128], idenb_t[:]
                    )
                    stg = stgpool.tile([128, D], dt.bfloat16, tag="stgb")
                    nc.scalar.activation(stg[:], pt[:], AF.Copy)
                    nc.sync.dma_start(
                        block_q[q][o + so: o + so + 128, 0:D], stg[:]
                    )
            nc.gpsimd.collective_compute(
                "AllGather",
                mybir.AluOpType.bypass,
                replica_groups=[list(range(NCORES))],
                ins=[block_q[q][:].opt()],
                outs=[table_q[par][q][:].opt()],
            )

        def emit_update(layer, ch):
            """Update MLP for one 512-col window (aggr complete for it)."""
            xT = xA
            xN = xA
            o = 512 * ch
            wd = 512
            p1 = ps_m16.tile([H, 512], dt.float32)
            nc.tensor.matmul(
                p1[:, :wd], lhsT=wu1a_t[layer][:], rhs=aggr[:, o: o + wd],
                start=True, stop=False, skip_group_check=True,
            )
            nc.tensor.matmul(
                p1[:, :wd], lhsT=wu1b_t[layer][:], rhs=xT[:, o: o + wd],
                start=False, stop=True, skip_group_check=True,
            )
            hid = hpool.tile([H, 512], dt.bfloat16, tag="hid")
            nc.scalar.activation(
                hid[:, :wd], p1[:, :wd], AF.Relu, bias=bu1_t[layer][:]
            )
            p2 = ps_m64.tile([D, 512], dt.float32)
            nc.tensor.matmul(
                p2[:, :wd], lhsT=wu2_t[layer][:], rhs=hid[:, :wd],
                start=True, stop=True,
            )
            if layer < N_LAYERS - 1:
                nc.scalar.activation(
                    xN[:, o: o + wd], p2[:, :wd], AF.Relu,
                    bias=bu2_t[layer][:],
                )
            else:
                xo = xopool.tile([D, 512], dt.float32)
                nc.scalar.activation(
                    xo[:, :wd], p2[:, :wd], AF.Relu, bias=bu2_t[layer][:]
                )
                nc.sync.dma_start(out_d[:, o: o + wd], xo[:, :wd])

        # window-pair -> quarter whose x columns are complete after it
        pdone = {3: 0, 6: 1, 9: 2, 12: 3}

        # prologue: load x0^T per quarter (host-transposed), cast to bf16
        for q in range(NQTR):
            for h in range(4):
                o = q * QP + h * 800
                xld = xldpool.tile([D, 800], dt.float32)
                nc.sync.dma_start(xld[:], x0_d[:, o: o + 800])
                nc.scalar.activation(xA[:, o: o + 800], xld[:], AF.Copy)
            msg_mlp_quarter(0, q)

        gcall = [0]

        for layer in range(KLAYERS):
            cursor = 0
            for (k, tiles, action) in blocks:
                ntiles = len(tiles)
                cur_win = -1
                pw = None
                tb = 0
                while tb < ntiles:
                    nt = min(SUB_TILES, ntiles - tb)
                    g = gpool.tile([128, SUB_TILES, 128], dt.bfloat16)
                    nidx = nt * 128
                    c0 = cursor + tb
                    idx_sl = idx_res[:, 8 * c0: 8 * (c0 + nt)]
                    nc.gpsimd.dma_gather(
                        g[:, :nt, :], table_q[layer % 2][k][:], idx_sl,
                        nidx, nidx, 128,
                        single_packet=False,
                        queue_num=gcall[0] % NQ,
                    )
                    gcall[0] += 1
                    sb = spool.tile([128, SUB_TILES, 128], dt.bfloat16)
                    io_b = iota_t[:].unsqueeze(1).broadcast_to([128, nt, 128])
                    sl_b = (
                        slot_t[:, c0: c0 + nt]
                        .unsqueeze(2)
                        .broadcast_to([128, nt, 128])
                    )
                    nc.vector.tensor_tensor(sb[:, :nt, :], io_b, sl_b, ALU.is_equal)

                    for j in range(nt):
                        (_kk, sw, first, last, _tix) = tiles[tb + j]
                        win = sw // 4
                        if win != cur_win:
                            if pw is not None:
                                _evac(nc, ALU, aggr, pw, cur_win, k)
                            pw = ps_seg.tile([D, 512], dt.float32)
                            cur_win = win
                        nc.tensor.matmul(
                            pw[:, 128 * (sw % 4): 128 * (sw % 4) + 128],
                            lhsT=g[:, j, 0:D],
                            rhs=sb[:, j, :],
                            start=first,
                            stop=last,
                        )
                    tb += nt
                if pw is not None:
                    _evac(nc, ALU, aggr, pw, cur_win, k)
                cursor += ntiles
                if action is not None:
                    p = action
                    emit_update(layer, 2 * p)
                    if p < 12:
                        emit_update(layer, 2 * p + 1)
                    if layer < KLAYERS - 1 and p in pdone:
                        msg_mlp_quarter(layer + 1, pdone[p])

    nc.finalize()
    return nc


def _evac(nc, ALU, aggr, pw, win, k):
    o = 512 * win
    wd = min(512, BPD - o)
    if k == 0:
        nc.vector.tensor_copy(aggr[:, o: o + wd], pw[:, :wd])
    else:
        nc.vector.tensor_tensor(
            aggr[:, o: o + wd], pw[:, :wd], aggr[:, o: o + wd], ALU.add
        )


_CACHE = {}


def _make_in_maps(inputs, idx_all, slot_all, pos):
    x = np.asarray(inputs["x"], dtype=np.float32)
    Wm1, bm1 = inputs["Wm1"], inputs["bm1"]
    Wm2, bm2 = inputs["Wm2"], inputs["bm2"]
    Wu1, bu1 = inputs["Wu1"], inputs["bu1"]
    Wu2, bu2 = inputs["Wu2"], inputs["bu2"]

    xpad = np.zeros((NCORES, BPD, D), dtype=np.float32)
    for c in range(NCORES):
        nodes = np.arange(c * B, (c + 1) * B)
        xpad[c, pos[nodes]] = x[nodes]
    xpadT = np.ascontiguousarray(xpad.transpose(0, 2, 1))

    iota = np.broadcast_to(np.arange(128, dtype=np.float32), (128, 128)).astype(_BF16)
    idenf = np.eye(128, dtype=np.float32)
    idenb = np.eye(64, dtype=np.float32).astype(_BF16)

    com = {
        "wm1": np.ascontiguousarray(np.asarray(Wm1, np.float32)).astype(_BF16),
        "bm1": np.asarray(bm1, np.float32).reshape(N_LAYERS, H, 1),
        "wm2": np.ascontiguousarray(np.asarray(Wm2, np.float32)).astype(_BF16),
        "bm2": np.asarray(bm2, np.float32).reshape(N_LAYERS, D, 1),
        "wu1a": np.ascontiguousarray(np.asarray(Wu1, np.float32)[:, :D, :]),
        "wu1b": np.ascontiguousarray(np.asarray(Wu1, np.float32)[:, D:, :]).astype(
            _BF16
        ),
        "bu1": np.asarray(bu1, np.float32).reshape(N_LAYERS, H, 1),
        "wu2": np.ascontiguousarray(np.asarray(Wu2, np.float32)).astype(_BF16),
        "bu2": np.asarray(bu2, np.float32).reshape(N_LAYERS, D, 1),
        "iota": iota,
        "idenf": idenf,
        "idenb": idenb,
    }
    in_maps = []
    for c in range(NCORES):
        m = dict(com)
        m["x0"] = xpadT[c]
        m["idx"] = np.ascontiguousarray(idx_all[c])
        m["slot"] = np.ascontiguousarray(slot_all[c])
        in_maps.append(m)
    return in_maps


def kernel(x, edge_index, Wm1, bm1, Wm2, bm2, Wu1, bu1, Wu2, bu2):
    from concourse.bass_utils import run_bass_kernel_spmd

    ei_key = hash(np.asarray(edge_index)[:, ::97].tobytes())
    if ei_key not in _CACHE:
        T, blocks, idx_all, slot_all, pos = _preprocess(edge_index)
        nc = _build_program(T, blocks)
        _CACHE[ei_key] = (nc, idx_all, slot_all, pos)
    nc, idx_all, slot_all, pos = _CACHE[ei_key]

    in_maps = _make_in_maps(
        dict(
            x=x, edge_index=edge_index, Wm1=Wm1, bm1=bm1, Wm2=Wm2, bm2=bm2,
            Wu1=Wu1, bu1=bu1, Wu2=Wu2, bu2=bu2,
        ),
        idx_all,
        slot_all,
        pos,
    )
    res = run_bass_kernel_spmd(nc, in_maps, core_ids=list(range(NCORES)))
    out = np.empty((N_NODES, D), dtype=np.float32)
    for c in range(NCORES):
        nodes = np.arange(c * B, (c + 1) * B)
        out[nodes] = res.results[c]["out"].T[pos[nodes]]
    return out


if __name__ == "__main__":
    import reference

    inputs = reference.setup_inputs()
    inputs = {k: np.asarray(v) for k, v in inputs.items()}
    got = kernel(**inputs)
    exp = np.asarray(reference.reference(**{k: v for k, v in inputs.items()}))
    err = np.abs(got - exp)
    rel = np.linalg.norm(got - exp) / np.linalg.norm(exp)
    print("max abs err:", err.max(), "rel:", rel)
